# revision 1
# baseline (speedup 1.0000x reference)
"""ECE (expected calibration error) kernel for Trainium2, 8 NeuronCores.

Math (matches torch ECELoss(n_bins=20) / the jax reference):
    conf_i = max_c outputs[i, c]
    acc_i  = 1[outputs[i, labels_i] == conf_i]   (== argmax correct; exact on
             this data - verified zero tie mismatches)
    bin membership via step functions S[i, b] = conf_i > b/20, b = 0..20
    cum[b] = sum_i S[i,b] * v_i  for v in {conf, acc}
    sum_v[b] = cum[b] - cum[b+1]         (equal-width (lo, hi] bins + clip)
    ece = sum_b |sum_conf[b] - sum_acc[b]| / N

Device mapping (per core, data-parallel over samples):
    - input arranged [P=128 partitions, JR rows, C=128 classes]; tile = 128
      samples x 128 classes; groups of G tiles per DMA (contiguous per
      partition).
    - VectorE: batched reduce_max over a group -> conf; per tile one
      scalar_tensor_tensor (iota == label) * x with accum_out -> picked =
      x[i, label] in a single pass (STT only exists on VectorE here).
    - GPSIMD: acc = (picked == conf) and S[i,b] = (conf > edge_b), each as
      TT-subtract + TS-compare-vs-0 (Pool TT comparisons don't lower on
      this toolchain; fp32 subtraction is sign-exact so this is identical).
    - TensorE: per-jumbo matmul [K=128] x ([2J] x [J*(NB+1)]) accumulating
      cum partial sums into PSUM across the whole shard.
    - host: sum the 8 cores' [2J, J*(NB+1)] partials, undo the jumbo
      cross-product layout, finish the 21->20 differencing and |.|/N.
    Measured ~360 us per core-shard pass (65.5 MB/core read) vs the ~183 us
    per-core HBM roofline, with VectorE (conf pass + 20 STT gathers) the
    bottleneck engine.
Padding rows are all-zero => conf = 0 => S == 0 => they contribute nothing.

Built on bacc.Bacc (not raw Bass): its compile pipeline legalizes
multi-sync-wait instructions via event semaphores, which this walrus build
requires (each ISA struct carries only one sync wait).
"""

import numpy as np

P = 128          # SBUF partitions (samples per tile)
C = 128          # classes
NB = 20          # ECE bins
NE = NB + 1      # bin edges
NCORES = 8
G = 20           # tiles per group (per DMA / per batched vector op)
                 # (G=40 measured: correct but slower — bigger x tiles hurt
                 # SBUF overlap more than the halved fixed costs help)
J = 10           # tiles per jumbo matmul (M = 2*J <= 128, N = J*NE <= 512)
N_DVE = 6        # how many of the G picked-gathers run on VectorE (rest GPSIMD)


def _get_winop():
    """Register (once) a custom DVE op: out = (C0 <= Idx < C1) * Src0,
    accum_out = sum(out). Single tensor input -> eligible for the fp32
    2x perf mode, unlike the two-input scalar_tensor_tensor gather."""
    import concourse.dve_ops as dvo

    for op in dvo.OPS:
        if op.name == "TENSOR_WINDOW_SUM_ANT":
            return op
    from operator import add

    import numpy as np_
    from concourse.dve_spec import C0, C1, Idx, Spec, Src0, Zero

    def ref(in0, in1, c0, c1, c2):
        p = in0.shape[0]
        x = in0.astype(np_.float32).reshape(p, -1)
        idx = np_.broadcast_to(
            np_.arange(x.shape[1], dtype=np_.float32), x.shape
        )
        b = (((idx >= c0) & (idx < c1)).astype(np_.float32) * x).astype(
            np_.float32
        )
        return b, b.sum(axis=-1, keepdims=True)

    op = dvo.DveOp(
        "TENSOR_WINDOW_SUM_ANT",
        Spec(
            body=((Idx >= C0) & (Idx < C1)) * Src0,
            accum=add,
            accum_init=Zero,
            reference=ref,
        ),
        subdim=False,
        uops_sha={"v3": "643c66c31669334b"},
        perf_en={"v3": True},
    )
    dvo.OPS.append(op)
    dvo._SUB_OPCODE_FOR_NAME[op.name] = (
        max(dvo._SUB_OPCODE_FOR_NAME.values()) + 1
    )
    dvo.CUSTOM_DVE_SPECS[op.name] = op.spec
    return op


def build_nc(jr, n_dve=N_DVE, repeat=1, do_stt=True, do_small=True,
             gather="stt"):
    """Build the Bass module for one core with JR rows per partition.

    repeat > 1 wraps the whole group loop in an on-device For_i that
    recomputes the same result `repeat` times (PSUM restarts each trip) —
    used only for wall-clock perf measurement via run-time deltas.
    """
    import contextlib

    import concourse.bacc as bacc
    import concourse.mybir as mybir
    from concourse.tile import TileContext

    f32 = mybir.dt.float32
    Alu = mybir.AluOpType
    ng = jr // G
    assert jr % G == 0 and G % J == 0
    nj = G // J

    nc = bacc.Bacc("TRN2", target_bir_lowering=False)
    x = nc.dram_tensor("x", (P, jr, C), f32, kind="ExternalInput")
    # one consts tensor = one DMA = one completion semaphore
    consts = nc.dram_tensor(
        "consts", (P, NE + C + jr), f32, kind="ExternalInput"
    )
    out = nc.dram_tensor("out", (2 * J, NE * J), f32, kind="ExternalOutput")

    with TileContext(nc) as tc:
        with (
            tc.tile_pool(name="consts", bufs=1) as cpool,
            tc.tile_pool(name="xin", bufs=4) as xpool,
            tc.tile_pool(name="vt", bufs=3) as vpool,
            tc.tile_pool(name="pk", bufs=3) as kpool,
            tc.tile_pool(name="st", bufs=3) as spool,
            tc.tile_pool(name="scrv", bufs=4) as scrvpool,
            tc.tile_pool(name="scrg", bufs=2) as scrgpool,
            tc.tile_pool(name="res", bufs=1) as rpool,
            tc.tile_pool(name="acc", bufs=1, space="PSUM") as ppool,
        ):
            constsb = cpool.tile([P, NE + C + jr], f32)
            nc.sync.dma_start(constsb[:], consts[:])
            edgesb = constsb[:][:, 0:NE]
            iotasb = constsb[:][:, NE:NE + C]
            labsb = constsb[:][:, NE + C:]
            if gather in ("tmr", "win"):
                # labels + 1 (window end)
                labp1 = cpool.tile([P, jr], f32)
                nc.vector.tensor_scalar_add(labp1[:], labsb, 1.0)
            winop = _get_winop() if gather == "win" else None

            psum = ppool.tile([2 * J, NE * J], f32)

            def group_body(g):
                xt = xpool.tile([P, G, C], f32)
                nc.sync.dma_start(xt[:], x[:, g * G:(g + 1) * G, :])

                # vt free layout: per jumbo j a contiguous [conf(J) | acc(J)]
                # block, so each matmul's stationary AP is one free dim.
                vt = vpool.tile([P, nj, 2 * J], f32)
                vt4 = vt[:].rearrange("p j (h t) -> p j h t", h=2)
                if not do_small:
                    nc.vector.memset(vt[:], 0.0)
                nc.vector.tensor_reduce(
                    vt4[:, :, 0, :], xt[:], axis=mybir.AxisListType.X, op=Alu.max
                )

                # picked[i, t] = x[i, label] : (iota == lab)*x, accum-summed.
                # STT only exists on VectorE (Pool fails the engine check).
                pk = kpool.tile([P, G], f32)
                for t in range(G if do_stt else 0):
                    scr = scrvpool.tile([P, C], f32)
                    if gather == "win":
                        nc.vector._custom_dve(
                            winop,
                            out=scr[:],
                            in0=xt[:][:, t, :],
                            s0=labsb[:, g * G + t: g * G + t + 1],
                            s1=labp1[:][:, g * G + t: g * G + t + 1],
                            accum_out=pk[:][:, t: t + 1],
                        )
                    elif gather == "tmr":
                        # picked = max over the [label, label+1) window
                        nc.vector.tensor_mask_reduce(
                            scr[:],
                            xt[:][:, t, :],
                            labsb[:, g * G + t: g * G + t + 1],
                            labp1[:][:, g * G + t: g * G + t + 1],
                            1.0,
                            -3.0e38,
                            Alu.max,
                            accum_out=pk[:][:, t: t + 1],
                        )
                    else:
                        nc.vector.scalar_tensor_tensor(
                            scr[:],
                            iotasb,
                            labsb[:, g * G + t: g * G + t + 1],
                            xt[:][:, t, :],
                            op0=Alu.is_equal,
                            op1=Alu.mult,
                            accum_out=pk[:][:, t: t + 1],
                        )

                # Pool: acc = (picked == conf), via subtract + compare-to-0
                # (Pool TT supports arithmetic ops only; TS supports cmp).
                # fp32 subtraction is sign-exact, so this matches is_equal.
                pk3 = pk[:].rearrange("p (j t) -> p j t", j=nj)
                st = spool.tile([P, G, NE], f32)
                st4 = st[:].rearrange("p (j t) e -> p j t e", j=nj)
                if not do_stt and do_small:
                    nc.vector.memset(pk[:], 0.0)
                if not do_small:
                    nc.vector.memset(st[:], 1.0)
                if do_small:
                    nc.gpsimd.tensor_tensor(
                        vt4[:, :, 1, :], pk3, vt4[:, :, 0, :], Alu.subtract
                    )
                    nc.gpsimd.tensor_scalar(
                        vt4[:, :, 1, :], vt4[:, :, 1, :], 0.0, None, Alu.is_equal
                    )

                    # Pool: S[i, t, b] = conf[i, t] > edge[b], same trick
                    conf4 = vt4[:, :, 0, :][:, :, :, None].broadcast_to(
                        [P, nj, J, NE]
                    )
                    edges4 = edgesb[:, None, None, :].broadcast_to(
                        [P, nj, J, NE]
                    )
                    nc.gpsimd.tensor_tensor(st4, conf4, edges4, Alu.subtract)
                    nc.gpsimd.tensor_scalar(st4, st4, 0.0, None, Alu.is_gt)

                # PE: accumulate cum[(h,t), (t',b)] += sum_i V[i,h,t]*S[i,t',b]
                for j in range(nj):
                    nc.tensor.matmul(
                        psum[:],
                        vt[:][:, j, :],
                        st[:][:, j * J:(j + 1) * J, :],
                        start=(g == 0 and j == 0),
                        stop=(g == ng - 1 and j == nj - 1),
                    )

            if repeat > 1:
                with tc.For_i(0, repeat, 1):
                    for g in range(ng):
                        group_body(g)
            else:
                for g in range(ng):
                    group_body(g)

            res = rpool.tile([2 * J, NE * J], f32)
            nc.scalar.copy(res[:], psum[:])
            nc.sync.dma_start(out[:], res[:])

    nc.finalize()
    return nc


def _prep_inputs(outputs, labels, ncores, jr):
    cap = ncores * P * jr
    n = outputs.shape[0]
    xpad = np.zeros((cap, C), np.float32)
    xpad[:n] = outputs
    lpad = np.zeros((cap,), np.float32)
    lpad[:n] = labels.astype(np.float32)
    xs = xpad.reshape(ncores, P, jr, C)
    ls = lpad.reshape(ncores, P, jr)
    consts = np.empty((ncores, P, NE + C + jr), np.float32)
    consts[:, :, 0:NE] = (np.arange(NE, dtype=np.float32) / NB).astype(
        np.float32
    )
    consts[:, :, NE:NE + C] = np.arange(C, dtype=np.float32)
    consts[:, :, NE + C:] = ls
    return [{"x": xs[c], "consts": consts[c]} for c in range(ncores)]


def _decode(core_outs, n):
    acc = np.zeros((2 * J, NE * J), np.float64)
    for r in core_outs:
        acc += r
    cum_conf = np.zeros(NE, np.float64)
    cum_acc = np.zeros(NE, np.float64)
    for k in range(J):
        cum_conf += acc[k, k * NE:(k + 1) * NE]
        cum_acc += acc[J + k, k * NE:(k + 1) * NE]
    sum_conf = cum_conf[:NB] - cum_conf[1:]
    sum_acc = cum_acc[:NB] - cum_acc[1:]
    ece = np.abs(sum_conf - sum_acc).sum() / n
    return np.array([ece], dtype=np.float32)


def kernel_impl(outputs, labels, trace=False):
    from concourse import bass_utils

    outputs = np.ascontiguousarray(np.asarray(outputs), dtype=np.float32)
    labels = np.asarray(labels)
    n = outputs.shape[0]
    assert outputs.shape[1] == C
    jr = -(-n // (NCORES * P * G)) * G  # ceil to a multiple of G
    nc = build_nc(jr)
    in_maps = _prep_inputs(outputs, labels, NCORES, jr)
    res = bass_utils.run_bass_kernel_spmd(
        nc, in_maps, core_ids=list(range(NCORES)), trace=trace
    )
    ece = _decode([r["out"] for r in res.results], n)
    return ece, res


def kernel(outputs, labels):
    ece, _ = kernel_impl(outputs, labels)
    return ece



# revision 6
# speedup vs baseline: 1.2094x; 1.2094x over previous
"""ECE (expected calibration error) kernel for Trainium2, 8 NeuronCores.

Math (matches torch ECELoss(n_bins=20) / the jax reference):
    conf_i = max_c outputs[i, c]
    acc_i  = 1[outputs[i, labels_i] == conf_i]   (== argmax correct; exact on
             this data - verified zero tie mismatches)
    bin membership via step functions S[i, b] = conf_i > b/20, b = 0..20
    cum[b] = sum_i S[i,b] * v_i  for v in {conf, acc}
    sum_v[b] = cum[b] - cum[b+1]         (equal-width (lo, hi] bins + clip)
    ece = sum_b |sum_conf[b] - sum_acc[b]| / N

This is memory-bound: the only full-data pass is the per-sample max.
Two levers get the kernel to the HBM roofline:
  1. x ships to device DRAM as bf16 (host cast) - halves HBM traffic.
     Numerically validated on the real data: ece rel-err 1.1e-3 vs the
     2e-2 gate (69/1M acc flips, 2387/1M bin moves).
  2. picked_i = outputs[i, labels_i] is gathered on the host (same O(N)
     prep pass that already pads/casts labels) and shipped in consts, so
     VectorE does no second full-data gather pass.

Device mapping (per core, data-parallel over samples):
    - input arranged [P=128 partitions, JR rows, C=128 classes] bf16;
      tile = 128 samples x 128 classes; groups of G tiles per DMA.
    - VectorE: 4-level pairwise tensor_tensor max tree in bf16 (2x_1P DVE
      mode, 2 elem/cycle) + a final 8-wide tensor_reduce -> conf (f32,
      exact bf16 upcast). Then one TT is_equal -> acc = (picked == conf).
      All comparisons are exact: bf16->f32 upcast is lossless and max
      selection never rounds.
    - GPSIMD: S[i,b] = (conf > edge_b) as TT-subtract + TS-compare-vs-0
      (Pool TT comparisons don't lower on this toolchain; fp32
      subtraction is sign-exact so this is identical).
    - TensorE: per-jumbo matmul [K=128] x ([2J] x [J*(NB+1)]) f32
      accumulating cum partial sums into PSUM across the whole shard.
    - host: sum the 8 cores' [2J, J*(NB+1)] partials, undo the jumbo
      cross-product layout, finish the 21->20 differencing and |.|/N.
Padding rows are all-zero => conf = 0 => S == 0 => they contribute nothing
(acc=1 on padding rows is harmless: acc only enters through S-weighted sums).

Built on bacc.Bacc (not raw Bass): its compile pipeline legalizes
multi-sync-wait instructions via event semaphores, which this walrus build
requires (each ISA struct carries only one sync wait).
"""

import numpy as np

P = 128          # SBUF partitions (samples per tile)
C = 128          # classes
NB = 20          # ECE bins
NE = NB + 1      # bin edges
NCORES = 8
G = 40           # tiles per group (per DMA / per batched vector op)
J = 10           # tiles per jumbo matmul (M = 2*J <= 128, N = J*NE <= 512)


def build_nc(jr, g=G, repeat=1, mode="host", xdt="bf16", tree=True,
             s_eng="gpsimd", do_max=True, do_small=True, dma_alt=False,
             xbufs=4):
    """Build the Bass module for one core with JR rows per partition.

    mode="host": picked (= x[i, label_i]) arrives precomputed in consts.
    mode="stt":  picked gathered on-device via VectorE STT (f32 fallback).
    xdt: dtype x is stored in device DRAM ("bf16" or "f32").
    tree: use the bf16 TT-max tree for conf (else one tensor_reduce).
    s_eng: engine for the S step functions ("gpsimd" or "vector").
    do_max/do_small: stage-isolation knobs for perf attribution.
    dma_alt: alternate x-group DMAs between the two HWDGE rings (SP/ACT).

    repeat > 1 wraps the whole group loop in an on-device For_i that
    recomputes the same result `repeat` times (PSUM restarts each trip) -
    used only for wall-clock perf measurement via run-time deltas.
    """
    import concourse.bacc as bacc
    import concourse.mybir as mybir
    from concourse.tile import TileContext

    f32 = mybir.dt.float32
    xd = mybir.dt.bfloat16 if xdt == "bf16" else f32
    if mode == "stt":
        assert xdt == "f32"
    if xdt == "f32":
        tree = False
    Alu = mybir.AluOpType
    ng = jr // g
    assert jr % g == 0 and g % J == 0
    nj = g // J

    nc = bacc.Bacc("TRN2", target_bir_lowering=False)
    x = nc.dram_tensor("x", (P, jr, C), xd, kind="ExternalInput")
    # one consts tensor = one DMA = one completion semaphore
    ncol = (NE + C + jr) if mode == "stt" else (NE + jr)
    consts = nc.dram_tensor("consts", (P, ncol), f32, kind="ExternalInput")
    out = nc.dram_tensor("out", (2 * J, NE * J), f32, kind="ExternalOutput")

    with TileContext(nc) as tc:
        with (
            tc.tile_pool(name="consts", bufs=1) as cpool,
            tc.tile_pool(name="xin", bufs=xbufs) as xpool,
            tc.tile_pool(name="mx", bufs=2) as mxpool,
            tc.tile_pool(name="vt", bufs=3) as vpool,
            tc.tile_pool(name="st", bufs=3) as spool,
            tc.tile_pool(name="pk", bufs=3) as kpool,
            tc.tile_pool(name="scrv", bufs=4) as scrvpool,
            tc.tile_pool(name="res", bufs=1) as rpool,
            tc.tile_pool(name="acc", bufs=1, space="PSUM") as ppool,
        ):
            constsb = cpool.tile([P, ncol], f32)
            nc.sync.dma_start(constsb[:], consts[:])
            edgesb = constsb[:][:, 0:NE]
            if mode == "stt":
                iotasb = constsb[:][:, NE:NE + C]
                pickb = constsb[:][:, NE + C:]
            else:
                pickb = constsb[:][:, NE:]

            psum = ppool.tile([2 * J, NE * J], f32)

            def group_body(gi):
                xt = xpool.tile([P, g, C], xd)
                eng = nc.scalar if (dma_alt and gi % 2) else nc.sync
                eng.dma_start(xt[:], x[:, gi * g:(gi + 1) * g, :])

                # vt free layout: per jumbo j a contiguous [conf(J) | acc(J)]
                # block, so each matmul's stationary AP is one free dim.
                vt = vpool.tile([P, nj, 2 * J], f32)
                vt4 = vt[:].rearrange("p j (h t) -> p j h t", h=2)
                if not do_max:
                    # perf isolation: keep a (tiny) real dependency on xt so
                    # the DMA is not dead-code-eliminated
                    nc.vector.tensor_reduce(
                        vt4[:, :, 0, :],
                        xt[:][:, :, 0:8],
                        axis=mybir.AxisListType.X,
                        op=Alu.max,
                    )
                elif tree:
                    m1 = mxpool.tile([P, g, 64], xd)
                    nc.vector.tensor_tensor(
                        m1[:], xt[:][:, :, 0:64], xt[:][:, :, 64:128], Alu.max
                    )
                    m2 = mxpool.tile([P, g, 32], xd)
                    nc.vector.tensor_tensor(
                        m2[:], m1[:][:, :, 0:32], m1[:][:, :, 32:64], Alu.max
                    )
                    m3 = mxpool.tile([P, g, 16], xd)
                    nc.vector.tensor_tensor(
                        m3[:], m2[:][:, :, 0:16], m2[:][:, :, 16:32], Alu.max
                    )
                    m4 = mxpool.tile([P, g, 8], xd)
                    nc.vector.tensor_tensor(
                        m4[:], m3[:][:, :, 0:8], m3[:][:, :, 8:16], Alu.max
                    )
                    m44 = m4[:].rearrange("p (j t) e -> p j t e", j=nj)
                    nc.vector.tensor_reduce(
                        vt4[:, :, 0, :], m44, axis=mybir.AxisListType.X,
                        op=Alu.max,
                    )
                else:
                    nc.vector.tensor_reduce(
                        vt4[:, :, 0, :], xt[:], axis=mybir.AxisListType.X,
                        op=Alu.max,
                    )

                if mode == "stt":
                    # picked[i, t] = x[i, label] : (iota == lab)*x, summed.
                    pk = kpool.tile([P, g], f32)
                    for t in range(g):
                        scr = scrvpool.tile([P, C], f32)
                        nc.vector.scalar_tensor_tensor(
                            scr[:],
                            iotasb,
                            pickb[:, gi * g + t: gi * g + t + 1],
                            xt[:][:, t, :],
                            op0=Alu.is_equal,
                            op1=Alu.mult,
                            accum_out=pk[:][:, t: t + 1],
                        )
                    pk3 = pk[:].rearrange("p (j t) -> p j t", j=nj)
                else:
                    pk3 = pickb[:, gi * g:(gi + 1) * g].rearrange(
                        "p (j t) -> p j t", j=nj
                    )

                st = spool.tile([P, g, NE], f32)
                st4 = st[:].rearrange("p (j t) e -> p j t e", j=nj)
                if not do_small:
                    nc.vector.memset(st[:], 1.0)
                    nc.vector.memset(vt4[:, :, 1, :], 0.0)
                else:
                    # VectorE: acc = (picked == conf); exact, see docstring
                    nc.vector.tensor_tensor(
                        vt4[:, :, 1, :], pk3, vt4[:, :, 0, :], Alu.is_equal
                    )

                    # S[i, t, b] = conf[i, t] > edge[b]
                    conf4 = vt4[:, :, 0, :][:, :, :, None].broadcast_to(
                        [P, nj, J, NE]
                    )
                    edges4 = edgesb[:, None, None, :].broadcast_to(
                        [P, nj, J, NE]
                    )
                    if s_eng == "vector":
                        nc.vector.tensor_tensor(st4, conf4, edges4, Alu.is_gt)
                    else:
                        # Pool TT cmp doesn't lower: subtract + TS cmp-vs-0
                        # (fp32 subtraction is sign-exact -> identical)
                        nc.gpsimd.tensor_tensor(
                            st4, conf4, edges4, Alu.subtract
                        )
                        nc.gpsimd.tensor_scalar(
                            st4, st4, 0.0, None, Alu.is_gt
                        )

                # PE: accumulate cum[(h,t), (t',b)] += sum_i V[i,h,t]*S[i,t',b]
                for j in range(nj):
                    nc.tensor.matmul(
                        psum[:],
                        vt[:][:, j, :],
                        st[:][:, j * J:(j + 1) * J, :],
                        start=(gi == 0 and j == 0),
                        stop=(gi == ng - 1 and j == nj - 1),
                    )

            if repeat > 1:
                with tc.For_i(0, repeat, 1):
                    for gi in range(ng):
                        group_body(gi)
            else:
                for gi in range(ng):
                    group_body(gi)

            res = rpool.tile([2 * J, NE * J], f32)
            nc.scalar.copy(res[:], psum[:])
            nc.sync.dma_start(out[:], res[:])

    nc.finalize()
    return nc


def _prep_inputs(outputs, labels, ncores, jr, mode="host", xdt="bf16"):
    import ml_dtypes

    cap = ncores * P * jr
    n = outputs.shape[0]
    npdt = ml_dtypes.bfloat16 if xdt == "bf16" else np.float32
    xpad = np.zeros((cap, C), npdt)
    xpad[:n] = outputs.astype(npdt)
    lpad = np.zeros((cap,), np.float32)
    if mode == "stt":
        lpad[:n] = labels.astype(np.float32)
    else:
        # gather from the (possibly rounded) shipped values so the device's
        # (picked == conf) equality is exact
        idx = np.asarray(labels).astype(np.int64)
        lpad[:n] = xpad[:n][np.arange(n), idx].astype(np.float32)
    xs = xpad.reshape(ncores, P, jr, C)
    ls = lpad.reshape(ncores, P, jr)
    ncol = (NE + C + jr) if mode == "stt" else (NE + jr)
    consts = np.empty((ncores, P, ncol), np.float32)
    consts[:, :, 0:NE] = (np.arange(NE, dtype=np.float32) / NB).astype(
        np.float32
    )
    if mode == "stt":
        consts[:, :, NE:NE + C] = np.arange(C, dtype=np.float32)
        consts[:, :, NE + C:] = ls
    else:
        consts[:, :, NE:] = ls
    return [{"x": xs[c], "consts": consts[c]} for c in range(ncores)]


def _decode(core_outs, n):
    acc = np.zeros((2 * J, NE * J), np.float64)
    for r in core_outs:
        acc += r
    cum_conf = np.zeros(NE, np.float64)
    cum_acc = np.zeros(NE, np.float64)
    for k in range(J):
        cum_conf += acc[k, k * NE:(k + 1) * NE]
        cum_acc += acc[J + k, k * NE:(k + 1) * NE]
    sum_conf = cum_conf[:NB] - cum_conf[1:]
    sum_acc = cum_acc[:NB] - cum_acc[1:]
    ece = np.abs(sum_conf - sum_acc).sum() / n
    return np.array([ece], dtype=np.float32)


def kernel_impl(outputs, labels, trace=False, g=G, mode="host", xdt="bf16",
                **build_kwargs):
    from concourse import bass_utils

    outputs = np.ascontiguousarray(np.asarray(outputs), dtype=np.float32)
    labels = np.asarray(labels)
    n = outputs.shape[0]
    assert outputs.shape[1] == C
    jr = -(-n // (NCORES * P * g)) * g  # ceil to a multiple of g
    nc = build_nc(jr, g=g, mode=mode, xdt=xdt, **build_kwargs)
    in_maps = _prep_inputs(outputs, labels, NCORES, jr, mode=mode, xdt=xdt)
    res = bass_utils.run_bass_kernel_spmd(
        nc, in_maps, core_ids=list(range(NCORES)), trace=trace
    )
    ece = _decode([r["out"] for r in res.results], n)
    return ece, res


def kernel(outputs, labels):
    ece, _ = kernel_impl(outputs, labels)
    return ece


# revision 19
# speedup vs baseline: 7.5255x; 6.2226x over previous
"""ECE (expected calibration error) kernel for Trainium2, 8 NeuronCores.

Math (matches torch ECELoss(n_bins=20) / the jax reference):
    conf_i = max_c outputs[i, c]
    acc_i  = 1[outputs[i, labels_i] == conf_i]   (== argmax correct; exact on
             this data - verified zero tie mismatches)
    bin membership via step functions S[i, b] = conf_i > b/20, b = 0..19
    cum[b] = sum_i S[i,b] * v_i  for v in {conf, acc}; cum[20] == 0 since
    conf <= 1 always
    sum_v[b] = cum[b] - cum[b+1]         (equal-width (lo, hi] bins + clip)
    ece = sum_b |sum_conf[b] - sum_acc[b]| / N

This is memory-bound: the only full-data pass is the per-sample max.
Measured stage floors on these cores (per-core shard = 32.8 MB bf16):
dma-only 94us, +max tree 108us. Design choices:
  1. x ships to device DRAM as bf16 (host cast) - halves HBM traffic.
     Validated on the real data: ece rel-err 1.1e-3 vs the 2e-2 gate.
  2. picked_i = outputs[i, labels_i] is gathered on the host (same O(N)
     prep pass that already pads/reshapes the inputs) and shipped in
     consts, so there is no second full-data gather pass on the device.
  3. GPSIMD is avoided entirely (measured 6+us per instruction here),
     and so is ScalarE for the step functions (per-op overhead measured
     slower than DVE's single broadcast compare; scheme="act"/"hybrid"
     keep that path available, with the sign<->step correction folded
     into the host decode).

Device mapping (per core, data-parallel over samples):
    - input [P=128 partitions, JR rows, C=128 classes] bf16; tile = 128
      samples x 128 classes; G=40 tiles per DMA group; K=5 groups per
      supergroup share one S/matmul batch to amortize ACT op overhead.
    - VectorE: 4-level pairwise tensor_tensor max tree in bf16 (2x_1P
      mode, 2 elem/cycle) + an 8-wide tensor_reduce -> conf (f32), then
      one TT is_equal -> acc = (picked == conf). Exact: bf16->f32 upcast
      is lossless and max selection never rounds.
    - VectorE also builds S = (conf > edge) as one broadcast TT is_gt
      per group (840 elems, 1x mode) plus one tiny TT is_equal for acc.
    - TensorE: per (group, jumbo) matmul [K=128] x ([2J] x [J*20]) f32
      accumulating cum_sign partials into PSUM across the whole shard.
    - host: sum the 8 cores' [2J, 20*J] partials, undo the layout, apply
      the sign->step correction, finish the 21->20 differencing and |.|/N.
Padding rows are all-zero => conf = 0 => handled exactly by the n_pad
correction above.

Built on bacc.Bacc (not raw Bass): its compile pipeline legalizes
multi-sync-wait instructions via event semaphores, which this walrus build
requires (each ISA struct carries only one sync wait).
"""

import numpy as np

P = 128          # SBUF partitions (samples per tile)
C = 128          # classes
NB = 20          # ECE bins == device edges (edge 1.0 dropped: cum[20]==0)
NCORES = 8
G = 50           # tiles per group (per DMA / per batched vector op)
K_SG = 4         # groups per supergroup (S/matmul batch)
J = 10           # tiles per jumbo matmul (M = 2*J <= 128, N = J*NB <= 512)


def build_nc(jr, g=G, k_sg=K_SG, repeat=1, scheme="vector", tree=True,
             l1c=1, do_max=True, do_small=True, xbufs=6, mxbufs=3,
             dma_alt=False, nored=False, perf_internal=False):
    """Build the Bass module for one core with JR rows per partition.

    scheme="act":    S = sign(conf-edge) on ScalarE (host decode corrects)
    scheme="vector": S = (conf > edge) via one DVE TT is_gt per supergroup
    tree/l1c: bf16 TT-max tree for conf; l1c chunks the first level.
    do_max/do_small: stage-isolation knobs for perf attribution.
    repeat > 1 wraps the loop in an on-device For_i recomputing the same
    result (PSUM restarts each trip) - for perf measurement via deltas.
    perf_internal: x becomes Internal DRAM (garbage data, no host
    transfer) - timing-only builds; runtime is data-independent.
    """
    import concourse.bacc as bacc
    import concourse.mybir as mybir
    from concourse.tile import TileContext

    f32 = mybir.dt.float32
    bf16 = mybir.dt.bfloat16
    Alu = mybir.AluOpType
    Act = mybir.ActivationFunctionType
    nsg = jr // (g * k_sg)
    assert jr % (g * k_sg) == 0 and g % J == 0
    nj = g // J

    nc = bacc.Bacc("TRN2", target_bir_lowering=False)
    xkind = "Internal" if perf_internal else "ExternalInput"
    x = nc.dram_tensor("x", (P, jr, C), bf16, kind=xkind)
    # consts: [-edges (act bias) | +edges (vector scheme) | picked]
    consts = nc.dram_tensor("consts", (P, 2 * NB + jr), f32,
                            kind="ExternalInput")
    out = nc.dram_tensor("out", (2 * J, NB * J), f32, kind="ExternalOutput")

    with TileContext(nc) as tc:
        with (
            tc.tile_pool(name="consts", bufs=1) as cpool,
            tc.tile_pool(name="xin", bufs=xbufs) as xpool,
            tc.tile_pool(name="mx", bufs=mxbufs) as mxpool,
            tc.tile_pool(name="va", bufs=2) as vapool,
            tc.tile_pool(name="st", bufs=2) as spool,
            tc.tile_pool(name="res", bufs=1) as rpool,
            tc.tile_pool(name="acc", bufs=1, space="PSUM") as ppool,
        ):
            constsb = cpool.tile([P, 2 * NB + jr], f32)
            nc.sync.dma_start(constsb[:], consts[:])
            negb = constsb[:][:, 0:NB]
            edgesb = constsb[:][:, NB:2 * NB]
            pickb = constsb[:][:, 2 * NB:]

            psum = ppool.tile([2 * J, NB * J], f32)

            def sg_body(sgi):
                # va free layout: per (k, j) a contiguous [conf(J) | acc(J)]
                # block, so each matmul's stationary AP is one free dim.
                va = vapool.tile([P, k_sg, nj, 2 * J], f32)
                va5 = va[:].rearrange("p k j (h t) -> p k j h t", h=2)
                st = spool.tile([P, k_sg, g, NB], f32)
                st5 = st[:].rearrange("p k (j t) e -> p k j t e", j=nj)

                for k in range(k_sg):
                    gi = sgi * k_sg + k
                    xt = xpool.tile([P, g, C], bf16)
                    deng = nc.scalar if (dma_alt and gi % 2) else nc.sync
                    deng.dma_start(xt[:], x[:, gi * g:(gi + 1) * g, :])

                    conf = va5[:, k, :, 0, :]
                    if not do_max:
                        # perf isolation: tiny real dependency on xt so the
                        # DMA is not dead-code-eliminated
                        nc.vector.tensor_reduce(
                            conf, xt[:][:, :, 0:8],
                            axis=mybir.AxisListType.X, op=Alu.max,
                        )
                    elif tree:
                        m1 = mxpool.tile([P, g, 64], bf16)
                        cg = g // l1c
                        for ci in range(l1c):
                            sl = slice(ci * cg, (ci + 1) * cg)
                            nc.vector.tensor_tensor(
                                m1[:][:, sl, :], xt[:][:, sl, 0:64],
                                xt[:][:, sl, 64:128], Alu.max
                            )
                        m2 = mxpool.tile([P, g, 32], bf16)
                        nc.vector.tensor_tensor(
                            m2[:], m1[:][:, :, 0:32], m1[:][:, :, 32:64],
                            Alu.max
                        )
                        m3 = mxpool.tile([P, g, 16], bf16)
                        nc.vector.tensor_tensor(
                            m3[:], m2[:][:, :, 0:16], m2[:][:, :, 16:32],
                            Alu.max
                        )
                        m4 = mxpool.tile([P, g, 8], bf16)
                        nc.vector.tensor_tensor(
                            m4[:], m3[:][:, :, 0:8], m3[:][:, :, 8:16],
                            Alu.max
                        )
                        if nored:
                            m5 = mxpool.tile([P, g, 4], bf16)
                            nc.vector.tensor_tensor(
                                m5[:], m4[:][:, :, 0:4], m4[:][:, :, 4:8],
                                Alu.max
                            )
                            m6 = mxpool.tile([P, g, 2], bf16)
                            nc.vector.tensor_tensor(
                                m6[:], m5[:][:, :, 0:2], m5[:][:, :, 2:4],
                                Alu.max
                            )
                            a6 = m6[:].rearrange("p (j t) e -> p j t e", j=nj)
                            nc.vector.tensor_tensor(
                                conf, a6[:, :, :, 0], a6[:, :, :, 1], Alu.max
                            )
                        else:
                            m44 = m4[:].rearrange(
                                "p (j t) e -> p j t e", j=nj
                            )
                            nc.vector.tensor_reduce(
                                conf, m44,
                                axis=mybir.AxisListType.X, op=Alu.max,
                            )
                    else:
                        nc.vector.tensor_reduce(
                            conf, xt[:],
                            axis=mybir.AxisListType.X, op=Alu.max,
                        )

                    # acc = (picked == conf); exact (see docstring)
                    pk3 = pickb[:, gi * g:(gi + 1) * g].rearrange(
                        "p (j t) -> p j t", j=nj
                    )
                    nc.vector.tensor_tensor(
                        va5[:, k, :, 1, :], pk3, conf, Alu.is_equal
                    )

                confs = va5[:, :, :, 0, :]
                if not do_small:
                    nc.vector.memset(st[:], 1.0)
                elif scheme == "act":
                    for b in range(NB):
                        nc.scalar.activation(
                            st5[:, :, :, :, b], confs, Act.Sign,
                            bias=negb[:, b:b + 1], scale=1.0,
                        )
                elif scheme == "hybrid":
                    # low half of the edges as (conf > e) on DVE, high half
                    # as sign(conf - e) on ScalarE (decode corrects those
                    # columns)
                    nh = NB // 2
                    for b in range(nh, NB):
                        nc.scalar.activation(
                            st5[:, :, :, :, b], confs, Act.Sign,
                            bias=negb[:, b:b + 1], scale=1.0,
                        )
                    edges4 = edgesb[:, None, None, 0:nh].broadcast_to(
                        [P, nj, J, nh]
                    )
                    for k in range(k_sg):
                        conf4 = va5[:, k, :, 0, :][:, :, :, None].broadcast_to(
                            [P, nj, J, nh]
                        )
                        nc.vector.tensor_tensor(
                            st5[:, k, :, :, 0:nh], conf4, edges4, Alu.is_gt
                        )
                else:
                    edges4 = edgesb[:, None, None, :].broadcast_to(
                        [P, nj, J, NB]
                    )
                    for k in range(k_sg):
                        conf4 = va5[:, k, :, 0, :][:, :, :, None].broadcast_to(
                            [P, nj, J, NB]
                        )
                        nc.vector.tensor_tensor(
                            st5[:, k], conf4, edges4, Alu.is_gt
                        )

                # PE: cum[(h,t), (t',b)] += sum_i V[i,h,t] * S[i,t',b]
                ng = jr // g
                for k in range(k_sg):
                    gi = sgi * k_sg + k
                    for j in range(nj):
                        nc.tensor.matmul(
                            psum[:],
                            va[:][:, k, j, :],
                            st[:][:, k, j * J:(j + 1) * J, :],
                            start=(gi == 0 and j == 0),
                            stop=(gi == ng - 1 and j == nj - 1),
                        )

            if repeat > 1:
                with tc.For_i(0, repeat, 1):
                    for sgi in range(nsg):
                        sg_body(sgi)
            else:
                for sgi in range(nsg):
                    sg_body(sgi)

            res = rpool.tile([2 * J, NB * J], f32)
            nc.scalar.copy(res[:], psum[:])
            nc.sync.dma_start(out[:], res[:])

    nc.finalize()
    return nc


def _prep_inputs(outputs, labels, ncores, jr):
    import ml_dtypes

    cap = ncores * P * jr
    n = outputs.shape[0]
    xpad = np.zeros((cap, C), ml_dtypes.bfloat16)
    xpad[:n] = outputs.astype(ml_dtypes.bfloat16)
    # gather from the rounded shipped values so (picked == conf) is exact
    lpad = np.zeros((cap,), np.float32)
    idx = np.asarray(labels).astype(np.int64)
    lpad[:n] = xpad[:n][np.arange(n), idx].astype(np.float32)
    xs = xpad.reshape(ncores, P, jr, C)
    ls = lpad.reshape(ncores, P, jr)
    consts = np.empty((ncores, P, 2 * NB + jr), np.float32)
    e = (np.arange(NB, dtype=np.float32) / NB).astype(np.float32)
    consts[:, :, 0:NB] = -e
    consts[:, :, NB:2 * NB] = e
    consts[:, :, 2 * NB:] = ls
    return [{"x": xs[c], "consts": consts[c]} for c in range(ncores)]


def _decode(core_outs, n, cap, scheme="act"):
    acc = np.zeros((2 * J, NB * J), np.float64)
    for r in core_outs:
        acc += r
    cum_conf = np.zeros(NB + 1, np.float64)
    cum_acc = np.zeros(NB + 1, np.float64)
    for k in range(J):
        cum_conf[:NB] += acc[k, k * NB:(k + 1) * NB]
        cum_acc[:NB] += acc[J + k, k * NB:(k + 1) * NB]
    if scheme in ("act", "hybrid"):
        # sign -> step correction (see module docstring); hybrid only uses
        # sign form for the high half of the edges
        lo = NB // 2 if scheme == "hybrid" else 1
        n_pad = cap - n
        tot_c = cum_conf[0]
        tot_a = cum_acc[0]
        cum_conf[lo:NB] = (cum_conf[lo:NB] + tot_c) / 2
        cum_acc[lo:NB] = (cum_acc[lo:NB] + tot_a + n_pad) / 2
    sum_conf = cum_conf[:NB] - cum_conf[1:]
    sum_acc = cum_acc[:NB] - cum_acc[1:]
    ece = np.abs(sum_conf - sum_acc).sum() / n
    return np.array([ece], dtype=np.float32)


def kernel_impl(outputs, labels, trace=False, g=G, k_sg=K_SG,
                scheme="vector", **build_kwargs):
    from concourse import bass_utils

    outputs = np.ascontiguousarray(np.asarray(outputs), dtype=np.float32)
    labels = np.asarray(labels)
    n = outputs.shape[0]
    assert outputs.shape[1] == C
    step = NCORES * P * g * k_sg
    jr = (-(-n // step) * step) // (NCORES * P)  # pad to full supergroups
    nc = build_nc(jr, g=g, k_sg=k_sg, scheme=scheme, **build_kwargs)
    in_maps = _prep_inputs(outputs, labels, NCORES, jr)
    res = bass_utils.run_bass_kernel_spmd(
        nc, in_maps, core_ids=list(range(NCORES)), trace=trace
    )
    ece = _decode([r["out"] for r in res.results], n, NCORES * P * jr,
                  scheme=scheme)
    return ece, res


def kernel(outputs, labels):
    ece, _ = kernel_impl(outputs, labels)
    return ece
